# revision 1
# baseline (speedup 1.0000x reference)
"""FlowNet correlation (kernel_size=1, max_displacement=4) on 8 Trainium2 cores.

Problem: input1, input2: [16, 256, 96, 96] fp32
         out[b, d, y, x] = (1/256) * sum_c in1[b,c,y,x] * in2pad[b,c,y+di,x+dj]
         d = (di+4)*9 + (dj+4), di,dj in [-4,4]  -> 81 output channels.

Sharding: data-parallel over batch, 2 samples per core, no collectives.

Per-core algorithm (per batch sample, per 8x16 pixel block):
  - inputs are DMA-cast fp32->bf16 into SBUF; in2 into a zero-padded
    [C, 104, 104] image so displaced reads never leave the tile.
  - TensorE: psum[m, n] = sum_c in1[c, m] * in2pad[c, n] with
      m = (yy, xx) over the 8x16 block        (M = 128)
      n = (ry, rx) over the 16x24 halo window (N = 384)
    as 2 accumulating bf16 matmuls (C = 2 x 128).
  - ScalarE/VectorE copy psum -> SBUF (bf16) with exact *2^-8 scaling.
  - The 81 correlation values of pixel m live at psum columns
    n = (yy+di)*24 + (xx+dj) = base(m) + di*24 + dj with base(m) =
    24*(m//16) + m%16 — a per-partition ("sheared") pattern no compute
    engine can address (engines broadcast one free-offset sequence to all
    lanes).  DMA descriptors *can* cross partitions, but only one AP dim
    may cross and its step must be partition-row-ALIGNED (fractional
    "diagonal" steps execute wrongly: the sub-row offset resets at every
    4-partition descriptor group boundary).  So the shear runs as two
    aligned hops over the contiguous 201-element window di*24+dj:
      hop a, 8 DMAs per group (one per yy):  +24*yy
      hop b, 16 DMAs per group (one per xx, stride-16 partition sets): +xx
    Both hops batch 12 blocks (2 by-rows x 6 bx) per DMA and split across
    the two HWDGE rings (SP + ACT).  The remaining gather
    sm[m, 201*c + 24*di + dj] is partition-uniform, so one engine copy
    compacts it to [128, 12*81] and a casting SWDGE DMA writes fp32 DRAM.
  - Host numpy reorders [b, byg, yy, xx, h, bx, di, dj] -> [b, d, y, x].
"""

import numpy as np

import concourse.bass as bass
import concourse.mybir as mybir
import concourse.tile as tile
from concourse import bacc
from concourse import bass_utils
import bass_rust

MD = 4
B, C, H, W = 16, 256, 96, 96
NCORES = 8
BPC = B // NCORES          # batches per core
KC = C // 128              # contraction chunks
PY, TX = 8, 16             # block: PY rows x TX cols = 128 output pixels
BY, BX = H // PY, W // TX  # 12 x 6 blocks
HP, WP = H + 2 * MD, W + 2 * MD  # padded in2: 104 x 104
WX = TX + 2 * MD           # window row width 24
NW = (PY + 2 * MD) * WX    # rhs window 16*24 = 384 columns
ND = (2 * MD + 1) ** 2     # 81 displacements
RUN = 2 * MD * WX + 2 * MD + 1  # 201: contiguous span covering di*24+dj
RA = RUN + TX - 1               # 216: hop-a run, covers xx + [0,201)
ROWCH = 16                 # input DMA row-chunk (rows per dma_start)

_cache = {}
DEBUG_DUMP = False


def _build(repeat: int = 1):
    f32 = mybir.dt.float32
    bf16 = mybir.dt.bfloat16
    nc = bacc.Bacc(None, target_bir_lowering=False, debug=False)

    in1_d = nc.dram_tensor("input1", [BPC, C, H, W], f32, kind="ExternalInput")
    in2_d = nc.dram_tensor("input2", [BPC, C, H, W], f32, kind="ExternalInput")
    out_d = nc.dram_tensor(
        "out", [BPC, BY // 2, 128 * 2 * BX * ND], f32, kind="ExternalOutput"
    )

    with tile.TileContext(nc) as tc:
        with (
            tc.tile_pool(name="inputs", bufs=1) as inp,
            tc.tile_pool(name="in1ch", bufs=2) as ch_pool,
            tc.tile_pool(name="dense", bufs=2) as dense_pool,
            tc.tile_pool(name="semi2", bufs=1) as semi2_pool,
            tc.tile_pool(name="semi", bufs=1) as semi_pool,
            tc.tile_pool(name="comp", bufs=2) as comp_pool,
            tc.tile_pool(name="psum", bufs=8, space="PSUM") as psum_pool,
        ):
            # in1 lives block-major so the (stationary) matmul operand is a
            # contiguous [128, 128] slice: free index = ((by*BX+bx)*PY+yy)*TX+xx
            in1_blk = {}
            in2_sb = {}
            for b in range(BPC):
                for k in range(KC):
                    in1_blk[b, k] = inp.tile(
                        [128, H * W], bf16, name=f"in1b_{b}_{k}", tag=f"in1b_{b}_{k}"
                    )
                    in2_sb[b, k] = inp.tile(
                        [128, HP * WP], bf16, name=f"in2_{b}_{k}", tag=f"in2_{b}_{k}"
                    )

            # zero the pad borders of the in2 tiles (the interior is fully
            # overwritten by the load below).
            for b in range(BPC):
                for k in range(KC):
                    v = in2_sb[b, k][:].rearrange("p (r c) -> p r c", r=HP)
                    nc.vector.memset(v[:, 0:MD, :], 0.0)
                    nc.vector.memset(v[:, HP - MD : HP, :], 0.0)
                    nc.vector.memset(v[:, MD : HP - MD, 0:MD], 0.0)
                    nc.vector.memset(v[:, MD : HP - MD, WP - MD : WP], 0.0)

            # input loads, fp32 -> bf16 cast on SWDGE, row-chunked so compute
            # can start before the whole image has landed.  in1 chunks are
            # re-tiled to block-major by an engine copy (DMA straight from
            # DRAM into block layout would need 64B descriptor rows).
            for _rep in range(repeat):
                cpy = 0
                for b in range(BPC):
                    for k in range(KC):
                        c0 = k * 128
                        for by in range(BY):
                            ch = ch_pool.tile([128, PY * W], bf16, tag="ch")
                            nc.gpsimd.dma_start(
                                ch[:],
                                in1_d[b, c0 : c0 + 128, by * PY : (by + 1) * PY, :],
                            )
                            src = ch[:].rearrange(
                                "p (y bx xx) -> p bx y xx", y=PY, bx=BX
                            )
                            dst = in1_blk[b, k][:, by * PY * W : (by + 1) * PY * W]
                            dst = dst.rearrange("p (bx y xx) -> p bx y xx", bx=BX, y=PY)
                            if cpy % 2 == 0:
                                nc.vector.tensor_copy(dst, src)
                            else:
                                nc.scalar.copy(dst, src)
                            cpy += 1
                        for r0 in range(0, H, ROWCH):
                            nc.gpsimd.dma_start(
                                in2_sb[b, k][:].rearrange("p (r c) -> p r c", r=HP)[
                                    :, MD + r0 : MD + r0 + ROWCH, MD : MD + W
                                ],
                                in2_d[b, c0 : c0 + 128, r0 : r0 + ROWCH, :],
                            )

                # block loop: by-rows of 6 bx-blocks; the de-shear and
                # output stages batch PAIRS of by-rows (GB=2) to halve the
                # HWDGE DMA count.  DMA access patterns allow exactly one
                # partition-crossing dim and fractional (diagonal) steps
                # mis-execute (offset resets every 4 partitions), so the shear
                # uses only partition-ALIGNED crossing dims.
                GB = 2
                B2 = GB * BX  # 12 blocks per batched shear group
                blk = 0
                for b in range(BPC):
                    for byg in range(BY // GB):
                        # s2g[m, (h*BX+bx)*RA + j] = dn[m, (h*BX+bx)*384 + 24*yy + j]
                        s2g = semi2_pool.tile([128, B2 * RA], bf16, tag="s2")
                        dn = dense_pool.tile([128, B2 * NW], bf16, tag="dn")
                        for h in range(GB):
                            by = byg * GB + h
                            for bx in range(BX):
                                ps = psum_pool.tile([128, NW], f32, tag="ps")
                                for k in range(KC):
                                    blkoff = (by * BX + bx) * PY * TX
                                    lhsT = in1_blk[b, k][:, blkoff : blkoff + PY * TX]
                                    rhs = in2_sb[b, k][:].rearrange(
                                        "p (r c) -> p r c", r=HP
                                    )[
                                        :,
                                        by * PY : by * PY + PY + 2 * MD,
                                        bx * TX : bx * TX + TX + 2 * MD,
                                    ]
                                    nc.tensor.matmul(
                                        ps[:], lhsT, rhs,
                                        start=(k == 0), stop=(k == KC - 1),
                                    )
                                c2 = h * BX + bx
                                dnb = dn[:, c2 * NW : (c2 + 1) * NW]
                                if blk % 2 == 0:
                                    nc.scalar.mul(dnb, ps[:], 1.0 / C)
                                else:
                                    nc.vector.tensor_scalar_mul(dnb, ps[:], 1.0 / C)
                                blk += 1

                        # hop a (+24*yy; per yy-group of 16 partitions):
                        for yy in range(PY):
                            sa = dn[:]
                            sa.ap = bass_rust.VecI64Pair(
                                [[B2 * NW, TX], [NW, B2], [1, RA]]
                            )
                            sa.offset = yy * TX * (B2 * NW) + WX * yy
                            da = s2g[:]
                            da.ap = bass_rust.VecI64Pair(
                                [[B2 * RA, TX], [RA, B2], [1, RA]]
                            )
                            da.offset = yy * TX * (B2 * RA)
                            (nc.scalar if yy % 2 else nc.sync).dma_start(da, sa)

                        # hop b (+xx; per xx-residue, stride-16 partition sets):
                        #   smg[m, c*201 + j] = s2g[m, c*216 + xx + j], c = h*BX+bx
                        smg = semi_pool.tile([128, B2 * RUN], bf16, tag="sm")
                        for xx in range(TX):
                            sb = s2g[:]
                            sb.ap = bass_rust.VecI64Pair(
                                [[TX * B2 * RA, PY], [RA, B2], [1, RUN]]
                            )
                            sb.offset = xx * (B2 * RA) + xx
                            db = smg[:]
                            db.ap = bass_rust.VecI64Pair(
                                [[TX * B2 * RUN, PY], [RUN, B2], [1, RUN]]
                            )
                            db.offset = xx * (B2 * RUN)
                            (nc.scalar if xx % 2 else nc.sync).dma_start(db, sb)

                        # partition-uniform gather of the 81 (di,dj) values
                        cpg = comp_pool.tile([128, B2 * ND], bf16, tag="cp")
                        gat = smg[:]
                        gat.ap = bass_rust.VecI64Pair(
                            [
                                [B2 * RUN, 128],
                                [RUN, B2],
                                [WX, 2 * MD + 1],
                                [1, 2 * MD + 1],
                            ]
                        )
                        cpv = cpg[:].rearrange(
                            "p (c di dj) -> p c di dj", c=B2, di=2 * MD + 1
                        )
                        if byg % 2 == 0:
                            nc.vector.tensor_copy(cpv, gat)
                        else:
                            nc.scalar.copy(cpv, gat)

                        # cast back to fp32 on the way out
                        nc.gpsimd.dma_start(out_d[b, byg, :], cpg[:])

            if DEBUG_DUMP:
                bf = mybir.dt.bfloat16
                d1 = nc.dram_tensor(
                    "dbg_in1blk", [128, H * W], bf, kind="ExternalOutput"
                )
                nc.sync.dma_start(d1[:], in1_blk[0, 0][:])
                d2_ = nc.dram_tensor(
                    "dbg_in2", [128, HP * WP], bf, kind="ExternalOutput"
                )
                nc.sync.dma_start(d2_[:], in2_sb[0, 0][:])

    nc.compile()
    return nc


def _make_runner(nc, n_cores=NCORES):
    """Replicate bass2jax.run_bass_via_pjrt's sharded executable, but reusable
    so repeated timed executions are possible (test harness only)."""
    import jax
    from jax.sharding import Mesh, PartitionSpec
    from jax.experimental.shard_map import shard_map
    import concourse.mybir as mybir
    from concourse import bass2jax

    bass2jax.install_neuronx_cc_hook()
    part_name = nc.partition_id_tensor.name if nc.partition_id_tensor else None
    in_names, out_names, out_avals, zero_outs = [], [], [], []
    for alloc in nc.m.functions[0].allocations:
        if not isinstance(alloc, mybir.MemoryLocationSet):
            continue
        name = alloc.memorylocations[0].name
        if alloc.kind == "ExternalInput":
            if name != part_name:
                in_names.append(name)
        elif alloc.kind == "ExternalOutput":
            out_names.append(name)
            shape = tuple(alloc.tensor_shape)
            dtype = mybir.dt.np(alloc.dtype)
            out_avals.append(jax.core.ShapedArray(shape, dtype))
            zero_outs.append(np.zeros(shape, dtype))
    n_params = len(in_names)
    n_outs = len(out_avals)
    all_names = in_names + out_names
    if part_name is not None:
        all_names = all_names + [part_name]

    def _body(*args):
        operands = list(args)
        if part_name is not None:
            operands.append(bass2jax.partition_id_tensor())
        outs = bass2jax._bass_exec_p.bind(
            *operands,
            out_avals=tuple(out_avals),
            in_names=tuple(all_names),
            out_names=tuple(out_names),
            lowering_input_output_aliases=(),
            sim_require_finite=True,
            sim_require_nnan=True,
            nc=nc,
        )
        return tuple(outs)

    devices = jax.devices()[:n_cores]
    mesh = Mesh(np.asarray(devices), ("core",))
    sharded = jax.jit(
        shard_map(
            _body,
            mesh=mesh,
            in_specs=(PartitionSpec("core"),) * (n_params + n_outs),
            out_specs=(PartitionSpec("core"),) * n_outs,
            check_rep=False,
        ),
        donate_argnums=tuple(range(n_params, n_params + n_outs)),
        keep_unused=True,
    )
    return sharded, in_names, out_names, zero_outs, mesh


def bench(input1: np.ndarray, input2: np.ndarray, iters: int = 12):
    """Return list of per-call wall times (s) for the full 8-core NEFF exec,
    with inputs already device-resident (measures dispatch + HW exec)."""
    import jax, time

    if "nc" not in _cache:
        _cache["nc"] = _build()
    sharded, in_names, out_names, zero_outs, mesh = _make_runner(_cache["nc"])
    from jax.sharding import NamedSharding, PartitionSpec

    shd = NamedSharding(mesh, PartitionSpec("core"))
    per_in = {"input1": input1, "input2": input2}
    concat_in = [np.ascontiguousarray(per_in[n], np.float32) for n in in_names]
    dev_in = [jax.device_put(a, shd) for a in concat_in]
    zsets = []
    for _ in range(iters):
        zsets.append(
            [
                jax.device_put(
                    np.zeros((NCORES * z.shape[0], *z.shape[1:]), z.dtype), shd
                )
                for z in zero_outs
            ]
        )
    # warmup (compiles + places inputs)
    out = sharded(*dev_in, *zsets.pop())
    jax.block_until_ready(out)
    times = []
    for zs in zsets:
        t0 = time.perf_counter()
        out = sharded(*dev_in, *zs)
        jax.block_until_ready(out)
        times.append(time.perf_counter() - t0)
    return times


def kernel(input1: np.ndarray, input2: np.ndarray) -> np.ndarray:
    input1 = np.ascontiguousarray(input1, dtype=np.float32)
    input2 = np.ascontiguousarray(input2, dtype=np.float32)
    if "nc" not in _cache:
        _cache["nc"] = _build()
    nc = _cache["nc"]

    in_maps = [
        {
            "input1": input1[i * BPC : (i + 1) * BPC],
            "input2": input2[i * BPC : (i + 1) * BPC],
        }
        for i in range(NCORES)
    ]
    res = bass_utils.run_bass_kernel_spmd(nc, in_maps, core_ids=list(range(NCORES)))
    _cache["last_results"] = res

    full = np.concatenate([r["out"] for r in res.results], axis=0)
    # device layout: [b, by, (yy, xx), bx, di, dj]
    # device layout: [b, byg, (yy, xx), (h, bx), di, dj]
    full = full.reshape(B, BY // 2, PY, TX, 2, BX, 2 * MD + 1, 2 * MD + 1)
    out = full.transpose(0, 6, 7, 1, 4, 2, 5, 3).reshape(B, ND, H, W)
    return np.ascontiguousarray(out)



# revision 5
# speedup vs baseline: 3.5640x; 3.5640x over previous
"""FlowNet correlation (kernel_size=1, max_displacement=4) on 8 Trainium2 cores.

Problem: input1, input2: [16, 256, 96, 96] fp32
         out[b, d, y, x] = (1/256) * sum_c in1[b,c,y,x] * in2pad[b,c,y+di,x+dj]
         d = (di+4)*9 + (dj+4), di,dj in [-4,4]  -> 81 output channels.

Sharding: data-parallel over batch, 2 samples per core, no collectives.

Per-core algorithm:
  - in2 is DMA-cast fp32->bf16 into flat [128, 96*96] SBUF tiles
    (2 contraction chunks) -- 4 large contiguous SWDGE DMAs.  in1 is
    DMA-cast in half-image chunks and engine-copied to block-major
    (the matmul's stationary operand must be a contiguous [128, 128]
    slice -- walrus checkMatmultInputs rejects strided lhsT).
  - Per 8x16 pixel block: TensorE psum[m, n] = sum_c in1[c, m] * in2[c, n]
    with m over the 128 block pixels (stationary) and n over the block's
    halo window CLAMPED to the image (<= 16x24 = 384 columns; smaller at
    image borders), read as a strided AP straight from the flat in2 tile.
    2 accumulating bf16 matmuls (C = 2 x 128).
  - ScalarE/VectorE copy psum -> a per-group SBUF staging tile (bf16).
  - One HWDGE DMA per group of 12 blocks writes the raw windows to DRAM
    (bf16).  No de-shear on device: the 81-of-window diagonal gather (a
    per-partition "sheared" pattern no engine can address and DMA does
    inefficiently) runs on the host, fully vectorized, together with the
    exact *2^-8 scaling, zero-fill of out-of-image displacements, and the
    layout transpose.
"""

import numpy as np

import concourse.bass as bass
import concourse.mybir as mybir
import concourse.tile as tile
from concourse import bacc
from concourse import bass_utils

MD = 4
B, C, H, W = 16, 256, 96, 96
NCORES = 8
BPC = B // NCORES          # batches per core
KC = C // 128              # contraction chunks
PY, TX = 8, 16             # block: PY rows x TX cols = 128 output pixels
BY, BX = H // PY, W // TX  # 12 x 6 blocks
GB = 2                     # by-rows per output group
NG = BY // GB              # 6 groups
ND = (2 * MD + 1) ** 2     # 81 displacements

# Per-image column layout of the clamped windows.
_BLK = {}        # (by, bx) -> (group, off within group, rv, cv, r0, c0)
_G_COLS = []     # columns per group
for _g in range(NG):
    _gc = 0
    for _h in range(GB):
        _by = _g * GB + _h
        for _bx in range(BX):
            _r0 = max(0, _by * PY - MD)
            _r1 = min(H, _by * PY + PY + MD)
            _c0 = max(0, _bx * TX - MD)
            _c1 = min(W, _bx * TX + TX + MD)
            _BLK[_by, _bx] = (_g, _gc, _r1 - _r0, _c1 - _c0, _r0, _c0)
            _gc += (_r1 - _r0) * (_c1 - _c0)
    _G_COLS.append(_gc)
_G_OFF = [sum(_G_COLS[:g]) for g in range(NG)]
TOT_COLS = sum(_G_COLS)    # 25024
GMAX = max(_G_COLS)        # 4352

_cache = {}


def _build(repeat: int = 1):
    f32 = mybir.dt.float32
    bf16 = mybir.dt.bfloat16
    nc = bacc.Bacc(None, target_bir_lowering=False, debug=False)

    in1_d = nc.dram_tensor("input1", [BPC, C, H, W], f32, kind="ExternalInput")
    in2_d = nc.dram_tensor("input2", [BPC, C, H, W], f32, kind="ExternalInput")
    out_d = nc.dram_tensor("out", [BPC, 128, TOT_COLS], bf16, kind="ExternalOutput")

    with tile.TileContext(nc) as tc:
        with (
            tc.tile_pool(name="inputs", bufs=1) as inp,
            tc.tile_pool(name="chunk", bufs=2) as ch_pool,
            tc.tile_pool(name="stage", bufs=3) as st_pool,
            tc.tile_pool(name="psum", bufs=8, space="PSUM") as psum_pool,
        ):
            HROWS = H // 2  # half-image chunk for in1 staging
            in1_blk, img2 = {}, {}
            for b in range(BPC):
                for k in range(KC):
                    in1_blk[b, k] = inp.tile(
                        [128, H * W], bf16, name=f"i1b_{b}_{k}", tag=f"i1b_{b}_{k}"
                    )
                    img2[b, k] = inp.tile(
                        [128, H * W], bf16, name=f"i2_{b}_{k}", tag=f"i2_{b}_{k}"
                    )

            for _rep in range(repeat):
                # large contiguous casting loads (SWDGE), batch-major so
                # batch 0 compute starts while batch 1 still streams in.
                # in1 is staged through half-image chunks and engine-copied
                # to block-major: free index ((by*BX+bx)*PY+yy)*TX+xx.
                cpy = 0
                for b in range(BPC):
                    for k in range(KC):
                        c0 = k * 128
                        nc.gpsimd.dma_start(img2[b, k][:], in2_d[b, c0:c0 + 128])
                        for half in range(2):
                            r0 = half * HROWS
                            ch = ch_pool.tile([128, HROWS * W], bf16, tag="ch")
                            nc.gpsimd.dma_start(
                                ch[:], in1_d[b, c0:c0 + 128, r0:r0 + HROWS, :]
                            )
                            chv = ch[:].rearrange(
                                "p (y bx xx) -> p y bx xx", bx=BX, xx=TX
                            )
                            for by in range(half * BY // 2, (half + 1) * BY // 2):
                                src = chv[:, (by * PY - r0):(by * PY - r0 + PY)]
                                src = src.rearrange("p y bx xx -> p bx y xx")
                                dst = in1_blk[b, k][
                                    :, by * PY * W : (by + 1) * PY * W
                                ].rearrange("p (bx y xx) -> p bx y xx", bx=BX, y=PY)
                                if cpy % 2 == 0:
                                    nc.vector.tensor_copy(dst, src)
                                else:
                                    nc.scalar.copy(dst, src)
                                cpy += 1

                cnt = 0
                for b in range(BPC):
                    for g in range(NG):
                        stg = st_pool.tile([128, GMAX], bf16, tag="stg")
                        for h in range(GB):
                            by = g * GB + h
                            for bx in range(BX):
                                _, boff, rv, cv, r0, c0 = _BLK[by, bx]
                                n = rv * cv
                                ps = psum_pool.tile([128, 384], f32, tag="ps")
                                for k in range(KC):
                                    blkoff = (by * BX + bx) * PY * TX
                                    lhsT = in1_blk[b, k][
                                        :, blkoff : blkoff + PY * TX
                                    ]
                                    v2 = img2[b, k][:].rearrange(
                                        "p (y x) -> p y x", y=H
                                    )
                                    rhs = v2[:, r0 : r0 + rv, c0 : c0 + cv]
                                    nc.tensor.matmul(
                                        ps[:, 0:n], lhsT, rhs,
                                        start=(k == 0), stop=(k == KC - 1),
                                    )
                                dst = stg[:, boff : boff + n]
                                if cnt % 2 == 0:
                                    nc.scalar.copy(dst, ps[:, 0:n])
                                else:
                                    nc.vector.tensor_copy(dst, ps[:, 0:n])
                                cnt += 1
                        gcols = _G_COLS[g]
                        nc.sync.dma_start(
                            out_d[b, :, _G_OFF[g] : _G_OFF[g] + gcols],
                            stg[:, 0:gcols],
                        )

    nc.compile()
    return nc


def _gather_tables():
    """Host gather indices: out[b, d, y, x] = dev[b, P[y, x], COL[d, y, x]]
    (masked).  dev is the device's [128, TOT_COLS] window dump per batch."""
    if "tables" in _cache:
        return _cache["tables"]
    yy, xx = np.meshgrid(np.arange(H), np.arange(W), indexing="ij")
    P = (yy % PY) * TX + (xx % TX)  # [96, 96]
    COL = np.zeros((ND, H, W), dtype=np.int64)
    MASK = np.zeros((ND, H, W), dtype=bool)
    goff_arr = np.zeros((H, W), dtype=np.int64)
    boff_arr = np.zeros((H, W), dtype=np.int64)
    cv_arr = np.zeros((H, W), dtype=np.int64)
    r0_arr = np.zeros((H, W), dtype=np.int64)
    c0_arr = np.zeros((H, W), dtype=np.int64)
    for by in range(BY):
        for bx in range(BX):
            g, boff, rv, cv, r0, c0 = _BLK[by, bx]
            sl = (slice(by * PY, (by + 1) * PY), slice(bx * TX, (bx + 1) * TX))
            goff_arr[sl] = _G_OFF[g]
            boff_arr[sl] = boff
            cv_arr[sl] = cv
            r0_arr[sl] = r0
            c0_arr[sl] = c0
    for di in range(-MD, MD + 1):
        for dj in range(-MD, MD + 1):
            d = (di + MD) * (2 * MD + 1) + (dj + MD)
            ry = yy + di
            rx = xx + dj
            ok = (ry >= 0) & (ry < H) & (rx >= 0) & (rx < W)
            col = goff_arr + boff_arr + (ry - r0_arr) * cv_arr + (rx - c0_arr)
            COL[d] = np.where(ok, col, 0)
            MASK[d] = ok
    _cache["tables"] = (P, COL, MASK)
    return _cache["tables"]


def kernel(input1: np.ndarray, input2: np.ndarray) -> np.ndarray:
    input1 = np.ascontiguousarray(input1, dtype=np.float32)
    input2 = np.ascontiguousarray(input2, dtype=np.float32)
    if "nc" not in _cache:
        _cache["nc"] = _build()
    nc = _cache["nc"]

    in_maps = [
        {
            "input1": input1[i * BPC : (i + 1) * BPC],
            "input2": input2[i * BPC : (i + 1) * BPC],
        }
        for i in range(NCORES)
    ]
    res = bass_utils.run_bass_kernel_spmd(nc, in_maps, core_ids=list(range(NCORES)))
    _cache["last_results"] = res

    dev = np.concatenate(
        [np.asarray(r["out"]).astype(np.float32) for r in res.results], axis=0
    )  # [B, 128, TOT_COLS]
    P, COL, MASK = _gather_tables()
    out = dev[:, P[np.newaxis, :, :], COL]  # [B, ND, H, W]
    out *= MASK
    out *= np.float32(1.0 / C)
    return np.ascontiguousarray(out, dtype=np.float32)


# revision 15
# speedup vs baseline: 3.7353x; 1.0481x over previous
"""FlowNet correlation (kernel_size=1, max_displacement=4) on 8 Trainium2 cores.

Problem: input1, input2: [16, 256, 96, 96] fp32
         out[b, d, y, x] = (1/256) * sum_c in1[b,c,y,x] * in2pad[b,c,y+di,x+dj]
         d = (di+4)*9 + (dj+4), di,dj in [-4,4]  -> 81 output channels.

Sharding: data-parallel over batch, 2 samples per core, no collectives.

Per-core algorithm:
  - in2 is DMA-cast fp32->bf16 into flat [128, 96*96] SBUF tiles
    (2 contraction chunks) -- 4 large contiguous SWDGE DMAs.  in1 is
    DMA-cast in half-image chunks and engine-copied to block-major
    (the matmul's stationary operand must be a contiguous [128, 128]
    slice -- walrus checkMatmultInputs rejects strided lhsT).
  - Per 8x16 pixel block: TensorE psum[m, n] = sum_c in1[c, m] * in2[c, n]
    with m over the 128 block pixels (stationary) and n over the block's
    halo window CLAMPED to the image (<= 16x24 = 384 columns; smaller at
    image borders), read as a strided AP straight from the flat in2 tile.
    2 accumulating bf16 matmuls (C = 2 x 128).
  - ScalarE/VectorE copy psum -> a per-group SBUF staging tile (bf16).
  - One HWDGE DMA per group of 12 blocks writes the raw windows to DRAM
    (bf16).  No de-shear on device: the 81-of-window diagonal gather (a
    per-partition "sheared" pattern no engine can address and DMA does
    inefficiently) runs on the host, fully vectorized, together with the
    exact *2^-8 scaling, zero-fill of out-of-image displacements, and the
    layout transpose.
"""

import numpy as np

import concourse.bass as bass
import concourse.mybir as mybir
import concourse.tile as tile
from concourse import bacc
from concourse import bass_utils
import bass_rust

MD = 4
B, C, H, W = 16, 256, 96, 96
NCORES = 8
BPC = B // NCORES          # batches per core
KC = C // 128              # contraction chunks
PY, TX = 8, 12             # block: PY rows x TX cols = 96 output pixels
BY, BX = H // PY, W // TX  # 12 x 6 blocks
GB = 2                     # by-rows per output group
NG = BY // GB              # 6 groups
ND = (2 * MD + 1) ** 2     # 81 displacements

# Per-image column layout of the clamped windows.
_BLK = {}        # (by, bx) -> (group, off within group, rv, cv, r0, c0)
_G_COLS = []     # columns per group
for _g in range(NG):
    _gc = 0
    for _h in range(GB):
        _by = _g * GB + _h
        for _bx in range(BX):
            _r0 = max(0, _by * PY - MD)
            _r1 = min(H, _by * PY + PY + MD)
            _c0 = max(0, _bx * TX - MD)
            _c1 = min(W, _bx * TX + TX + MD)
            _BLK[_by, _bx] = (_g, _gc, _r1 - _r0, _c1 - _c0, _r0, _c0)
            _gc += (_r1 - _r0) * (_c1 - _c0)
    _G_COLS.append(_gc)
_G_OFF = [sum(_G_COLS[:g]) for g in range(NG)]
TOT_COLS = sum(_G_COLS)    # 25024
GMAX = max(_G_COLS)        # 4352

_cache = {}


def _build(repeat: int = 1):
    f32 = mybir.dt.float32
    bf16 = mybir.dt.bfloat16
    nc = bacc.Bacc(None, target_bir_lowering=False, debug=False)

    in1_d = nc.dram_tensor("input1", [BPC, C, H, W], f32, kind="ExternalInput")
    in2_d = nc.dram_tensor("input2", [BPC, C, H, W], f32, kind="ExternalInput")
    out_d = nc.dram_tensor("out", [BPC, PY * TX, TOT_COLS], bf16, kind="ExternalOutput")

    with tile.TileContext(nc) as tc:
        with (
            tc.tile_pool(name="inputs", bufs=1) as inp,
            tc.tile_pool(name="chunk", bufs=2) as ch_pool,
            tc.tile_pool(name="stage", bufs=3) as st_pool,
            tc.tile_pool(name="psum", bufs=8, space="PSUM") as psum_pool,
        ):
            HROWS = H // 2  # half-image chunk for in1 staging
            in1_blk, img2 = {}, {}
            for b in range(BPC):
                for k in range(KC):
                    in1_blk[b, k] = inp.tile(
                        [128, H * W], bf16, name=f"i1b_{b}_{k}", tag=f"i1b_{b}_{k}"
                    )
                    img2[b, k] = inp.tile(
                        [128, H * W], bf16, name=f"i2_{b}_{k}", tag=f"i2_{b}_{k}"
                    )

            for _rep in range(repeat):
                # large contiguous casting loads (SWDGE), batch-major so
                # batch 0 compute starts while batch 1 still streams in.
                # in1 is staged through half-image chunks and engine-copied
                # to block-major: free index ((by*BX+bx)*PY+yy)*TX+xx.
                # loads are split into row-halves, ordered so each batch's
                # top-half groups become compute-ready while its bottom half
                # still streams in (keeps DMA_ENGINES saturated at the tail).
                # in2 splits at row 52 (group g2's halo needs rows up to 51).
                cpy = 0

                def load_in2(b, k, s0, s1):
                    c0 = k * 128
                    nc.gpsimd.dma_start(
                        img2[b, k][:, s0 * W : s1 * W],
                        in2_d[b, c0:c0 + 128, s0:s1, :],
                    )

                def load_in1(b, k, r0, r1):
                    nonlocal cpy
                    c0 = k * 128
                    ch = ch_pool.tile([128, 32 * W], bf16, tag="ch")
                    nc.gpsimd.dma_start(
                        ch[:, 0 : (r1 - r0) * W],
                        in1_d[b, c0:c0 + 128, r0:r1, :],
                    )
                    chv = ch[:, 0 : (r1 - r0) * W].rearrange(
                        "p (y bx xx) -> p y bx xx", bx=BX, xx=TX
                    )
                    for by in range(r0 // PY, r1 // PY):
                        src = chv[:, (by * PY - r0):(by * PY - r0 + PY)]
                        src = src.rearrange("p y bx xx -> p bx y xx")
                        dst = in1_blk[b, k][
                            :, by * PY * W : (by + 1) * PY * W
                        ].rearrange("p (bx y xx) -> p bx y xx", bx=BX, y=PY)
                        if cpy % 2 == 0:
                            nc.vector.tensor_copy(dst, src)
                        else:
                            nc.scalar.copy(dst, src)
                        cpy += 1

                # loads arrive in thirds (piece p enables groups 2p, 2p+1
                # of a batch: in1 rows < 32p+32, in2 halo rows < 36+32p), so
                # compute starts early and the out-DMA backlog stays ahead
                # of the drain.
                I1P = [(0, 32), (32, 64), (64, 96)]
                I2P = [(0, 36), (36, 68), (68, 96)]
                for p in range(3):
                    for b in range(BPC):
                        for k in range(KC):
                            load_in2(b, k, *I2P[p])
                            load_in1(b, k, *I1P[p])

                cnt = 0
                # group order matches load-piece arrival.
                SCHED = [(b, g) for gr in ((0, 1), (2, 3), (4, 5))
                         for b in range(BPC) for g in gr]
                for (b, g) in SCHED:
                    if True:
                        stg = st_pool.tile([PY * TX, GMAX], bf16, tag="stg")
                        for h in range(GB):
                            by = g * GB + h
                            for bx in range(BX):
                                _, boff, rv, cv, r0, c0 = _BLK[by, bx]
                                n = rv * cv
                                ps = psum_pool.tile([PY * TX, 512], f32, tag="ps")
                                for k in range(KC):
                                    blkoff = (by * BX + bx) * PY * TX
                                    lhsT = in1_blk[b, k][
                                        :, blkoff : blkoff + PY * TX
                                    ]
                                    v2 = img2[b, k][:].rearrange(
                                        "p (y x) -> p y x", y=H
                                    )
                                    rhs = v2[:, r0 : r0 + rv, c0 : c0 + cv]
                                    nc.tensor.matmul(
                                        ps[:, 0:n], lhsT, rhs,
                                        start=(k == 0), stop=(k == KC - 1),
                                    )
                                dst = stg[:, boff : boff + n]
                                if cnt % 2 == 0:
                                    nc.scalar.copy(dst, ps[:, 0:n])
                                else:
                                    nc.vector.tensor_copy(dst, ps[:, 0:n])
                                cnt += 1
                        gcols = _G_COLS[g]
                        nc.sync.dma_start(
                            out_d[b, :, _G_OFF[g] : _G_OFF[g] + gcols],
                            stg[:, 0:gcols],
                        )

    nc.compile()
    return nc


def _gather_tables():
    """Host gather indices: out[b, d, y, x] = dev[b, P[y, x], COL[d, y, x]]
    (masked).  dev is the device's [128, TOT_COLS] window dump per batch."""
    if "tables" in _cache:
        return _cache["tables"]
    yy, xx = np.meshgrid(np.arange(H), np.arange(W), indexing="ij")
    P = (yy % PY) * TX + (xx % TX)  # [96, 96]
    COL = np.zeros((ND, H, W), dtype=np.int64)
    MASK = np.zeros((ND, H, W), dtype=bool)
    goff_arr = np.zeros((H, W), dtype=np.int64)
    boff_arr = np.zeros((H, W), dtype=np.int64)
    cv_arr = np.zeros((H, W), dtype=np.int64)
    r0_arr = np.zeros((H, W), dtype=np.int64)
    c0_arr = np.zeros((H, W), dtype=np.int64)
    for by in range(BY):
        for bx in range(BX):
            g, boff, rv, cv, r0, c0 = _BLK[by, bx]
            sl = (slice(by * PY, (by + 1) * PY), slice(bx * TX, (bx + 1) * TX))
            goff_arr[sl] = _G_OFF[g]
            boff_arr[sl] = boff
            cv_arr[sl] = cv
            r0_arr[sl] = r0
            c0_arr[sl] = c0
    for di in range(-MD, MD + 1):
        for dj in range(-MD, MD + 1):
            d = (di + MD) * (2 * MD + 1) + (dj + MD)
            ry = yy + di
            rx = xx + dj
            ok = (ry >= 0) & (ry < H) & (rx >= 0) & (rx < W)
            col = goff_arr + boff_arr + (ry - r0_arr) * cv_arr + (rx - c0_arr)
            COL[d] = np.where(ok, col, 0)
            MASK[d] = ok
    _cache["tables"] = (P, COL, MASK)
    return _cache["tables"]


def kernel(input1: np.ndarray, input2: np.ndarray) -> np.ndarray:
    input1 = np.ascontiguousarray(input1, dtype=np.float32)
    input2 = np.ascontiguousarray(input2, dtype=np.float32)
    if "nc" not in _cache:
        _cache["nc"] = _build()
    nc = _cache["nc"]

    in_maps = [
        {
            "input1": input1[i * BPC : (i + 1) * BPC],
            "input2": input2[i * BPC : (i + 1) * BPC],
        }
        for i in range(NCORES)
    ]
    res = bass_utils.run_bass_kernel_spmd(nc, in_maps, core_ids=list(range(NCORES)))
    _cache["last_results"] = res

    dev = np.concatenate(
        [np.asarray(r["out"]).astype(np.float32) for r in res.results], axis=0
    )  # [B, 128, TOT_COLS]
    P, COL, MASK = _gather_tables()
    out = dev[:, P[np.newaxis, :, :], COL]  # [B, ND, H, W]
    out = np.where(MASK, out, np.float32(0.0))  # NaN-safe for x-halo garbage
    out *= np.float32(1.0 / C)
    return np.ascontiguousarray(out, dtype=np.float32)
